# revision 9
# baseline (speedup 1.0000x reference)
"""Trainium2 Bass kernel for nn_CLinear (group-quantized linear layer).

Computes out = x @ dequant(qweight).T + bias where
  x:       [4, 2048, 4096] f32
  qweight: [11008, 16, 256] int8 (group-quantized, G=256)
  scale:   [11008, 16, 1]   f32  (w = qweight / scale)
  bias:    [11008]          f32
  out:     [4, 2048, 11008] f32

Sharding: column-parallel (tensor-parallel over out_features) across 8
NeuronCores.  OUT is padded 11008 -> 11264 = 8 * 1408 so every core gets
11 full 128-row tiles.  x is replicated to every core.

All activation/weight preprocessing happens on the host: x is cast to
bf16 and laid out as K-major lhsT tiles xt[m, p, u, t] = x[128m+t, 128u+p];
the weight shard is dequantized to bf16 and transposed to
wt[p, u, o] = w[o, 128u+p].  The device kernel is then a pure streaming
GEMM: resident weights + bias, stream x tiles in, 32 accumulating bf16
matmuls per (token-tile, out-block) into PSUM f32 (k-chunk-outer order so
the PE only ever waits for the earliest unarrived weight chunk), DVE adds
bias on PSUM->SBUF evict, DMA the f32 result out.
"""

import numpy as np

import concourse.bass as bass
import concourse.mybir as mybir
import concourse.tile as tile
from concourse import bacc
from concourse.bass_utils import run_bass_kernel_spmd

P = 128
B, S, IN, OUT, G = 4, 2048, 4096, 11008, 256
NCORES = 8
T = B * S                      # 8192 tokens
OUT_PAD = ((OUT + NCORES * P - 1) // (NCORES * P)) * (NCORES * P)  # 11264
OUT_SH = OUT_PAD // NCORES     # 1408 out features per core
NG = IN // G                   # 16 quant groups per row
KT = IN // P                   # 32 k-tiles
MT = T // P                    # 64 token tiles
UCH = 2                        # k-tiles per resident weight chunk
XSPLIT = 4                     # sub-DMAs per x tile (finer PE wakeup)
XU = KT // XSPLIT              # k-tiles per x sub-tile
F32 = mybir.dt.float32
BF16 = mybir.dt.bfloat16

NBLK = [(0, 512), (512, 512), (1024, OUT_SH - 1024)]


def emit_kernel(tc, nc, xt_d, wt_d, bb_d, y_d):
    """Per-core kernel IR.

    xt_d: [MT, P, KT, P] bf16  (host-transposed lhsT tiles, replicated)
    wt_d: [P, KT, OUT_SH] bf16 (host-dequantized transposed weight shard)
    bb_d: [P, OUT_SH]     f32  (bias shard broadcast along partitions)
    y_d:  [T, OUT_SH]     f32  (output shard)
    """
    from contextlib import ExitStack
    ctx = ExitStack()
    const = ctx.enter_context(tc.tile_pool(name="const", bufs=1))
    wtp = ctx.enter_context(tc.tile_pool(name="wt", bufs=1))
    xp = ctx.enter_context(tc.tile_pool(name="x", bufs=4))
    outp = ctx.enter_context(tc.tile_pool(name="out", bufs=3))
    psp = ctx.enter_context(tc.tile_pool(name="psum", bufs=2, space="PSUM"))

    DEPTH = 2

    def produce(m):
        subs = []
        for i in range(XSPLIT):
            xt = xp.tile([P, XU, P], BF16, name=f"xt{i}")
            nc.sync.dma_start(xt[:], xt_d[m, :, i * XU:(i + 1) * XU, :])
            subs.append(xt)
        return subs

    # First x tiles ahead of the weight stream on their own queue.
    xts = {m: produce(m) for m in range(min(DEPTH, MT))}

    biasb = const.tile([P, OUT_SH], F32)
    nc.gpsimd.dma_start(biasb[:], bb_d[:, :])
    # Resident weight chunks; separate tiles so early matmuls only wait on
    # the first chunk.  Spread over three DMA queues.
    wts = []
    wq = [nc.scalar, nc.gpsimd, nc.sync]
    for g in range(KT // UCH):
        wtt = wtp.tile([P, UCH, OUT_SH], BF16, name=f"wt{g}")
        wq[g % 3].dma_start(wtt[:], wt_d[:, g * UCH:(g + 1) * UCH, :])
        wts.append(wtt)

    sq = [nc.gpsimd, nc.scalar, nc.sync]

    def evict(m, nb, n0, sz, ps, qi=None):
        t0 = m * P
        ot = outp.tile([P, 512], F32, name=f"ot{nb}")
        nc.vector.tensor_tensor(
            ot[:, :sz], ps, biasb[:, n0:n0 + sz], mybir.AluOpType.add
        )
        sq[(m if qi is None else qi) % 3].dma_start(
            y_d[t0:t0 + P, n0:n0 + sz], ot[:, :sz])

    def alloc_ps():
        return [psp.tile([P, 512], F32, name=f"ps{nb}")[:, :sz]
                for nb, (n0, sz) in enumerate(NBLK)]

    def mm(pss, xtf, u, nb, n0, sz):
        nc.tensor.matmul(
            pss[nb],
            xtf[u // XU][:, u % XU, :],
            wts[u // UCH][:, u % UCH, n0:n0 + sz],
            start=(u == 0),
            stop=(u == KT - 1),
        )

    # Startup: interleave the first two token tiles chunk-by-chunk (6 PSUM
    # banks) so the PE's weight-consumption rate stays below the DMA
    # delivery rate while the resident weight stream lands.
    NSTART = min(2, MT)
    for m in range(NSTART, min(NSTART + DEPTH, MT)):
        xts[m] = produce(m)
    spss = [alloc_ps() for _ in range(NSTART)]
    for u in range(KT):
        for i in range(NSTART):
            for nb, (n0, sz) in enumerate(NBLK):
                mm(spss[i], xts[i], u, nb, n0, sz)
    for i in range(NSTART):
        xts.pop(i)
        for nb, (n0, sz) in enumerate(NBLK):
            evict(i, nb, n0, sz, spss[i][nb])

    for m in range(NSTART, MT):
        if m + DEPTH < MT:
            xts[m + DEPTH] = produce(m + DEPTH)
        xtf = xts.pop(m)
        pss = alloc_ps()
        if m == MT - 1:
            # Last tile: block-sequential so each block's evict+store
            # overlaps the remaining blocks' matmuls, and stores rotated
            # across queues so the final drain flushes in parallel.
            for nb, (n0, sz) in enumerate(NBLK):
                for u in range(KT):
                    mm(pss, xtf, u, nb, n0, sz)
                evict(m, nb, n0, sz, pss[nb], qi=m + nb)
        else:
            # k-chunk-outer so weight chunks are consumed in arrival order.
            for u in range(KT):
                for nb, (n0, sz) in enumerate(NBLK):
                    mm(pss, xtf, u, nb, n0, sz)
            for nb, (n0, sz) in enumerate(NBLK):
                evict(m, nb, n0, sz, pss[nb])

    ctx.close()


def build_nc(debug=False):
    nc = bacc.Bacc(
        "TRN2",
        target_bir_lowering=False,
        debug=debug,
        num_devices=NCORES,
        enable_asserts=debug,
    )
    xt_d = nc.dram_tensor("xt", [MT, P, KT, P], BF16, kind="ExternalInput").ap()
    wt_d = nc.dram_tensor("wt", [P, KT, OUT_SH], BF16, kind="ExternalInput").ap()
    bb_d = nc.dram_tensor("biasb", [P, OUT_SH], F32, kind="ExternalInput").ap()
    y_d = nc.dram_tensor("y", [T, OUT_SH], F32, kind="ExternalOutput").ap()
    with tile.TileContext(nc) as tc:
        emit_kernel(tc, nc, xt_d, wt_d, bb_d, y_d)
    nc.compile()
    return nc


_NC_CACHE = {}


def _get_nc():
    if "nc" not in _NC_CACHE:
        _NC_CACHE["nc"] = build_nc()
    return _NC_CACHE["nc"]


def prep_inputs(x, qweight, scale, bias):
    """Host-side prep. Returns in_maps for run_bass_kernel_spmd."""
    import ml_dtypes
    x = np.asarray(x)
    qw = np.asarray(qweight)
    sc = np.asarray(scale, dtype=np.float32)
    b = np.asarray(bias, dtype=np.float32)

    # xt[m, p, u, t] = x[128m + t, 128u + p], bf16
    x2 = x.reshape(T, IN).astype(ml_dtypes.bfloat16)
    xt = np.ascontiguousarray(
        x2.reshape(MT, P, KT, P).transpose(0, 3, 2, 1))

    # Dequantize exactly as the reference does (q / scale, f32), then bf16.
    qw2 = qw.reshape(OUT, NG, G)
    w = (qw2.astype(np.float32) / sc.reshape(OUT, NG, 1)).reshape(OUT, IN)
    w_p = np.zeros((OUT_PAD, IN), dtype=ml_dtypes.bfloat16)
    w_p[:OUT] = w.astype(ml_dtypes.bfloat16)
    b_p = np.zeros(OUT_PAD, dtype=np.float32)
    b_p[:OUT] = b

    in_maps = []
    for c in range(NCORES):
        sl = slice(c * OUT_SH, (c + 1) * OUT_SH)
        # wt[p, u, o] = w[o, 128u + p]
        wt = np.ascontiguousarray(
            w_p[sl].reshape(OUT_SH, KT, P).transpose(2, 1, 0))
        in_maps.append({
            "xt": xt,
            "wt": wt,
            "biasb": np.ascontiguousarray(
                np.broadcast_to(b_p[sl][None, :], (P, OUT_SH))
            ),
        })
    return in_maps


def run(x, qweight, scale, bias, trace=False):
    nc = _get_nc()
    in_maps = prep_inputs(x, qweight, scale, bias)
    res = run_bass_kernel_spmd(nc, in_maps, core_ids=list(range(NCORES)),
                               trace=trace)
    ys = [np.asarray(res.results[c]["y"]) for c in range(NCORES)]
    out = np.concatenate(ys, axis=1)[:, :OUT]
    return out.reshape(B, S, OUT).astype(np.float32, copy=False), res


def kernel(x, qweight, scale, bias):
    out, _ = run(x, qweight, scale, bias, trace=False)
    return out


# revision 11
# speedup vs baseline: 1.0028x; 1.0028x over previous
"""Trainium2 Bass kernel for nn_CLinear (group-quantized linear layer).

Computes out = x @ dequant(qweight).T + bias where
  x:       [4, 2048, 4096] f32
  qweight: [11008, 16, 256] int8 (group-quantized, G=256)
  scale:   [11008, 16, 1]   f32  (w = qweight / scale)
  bias:    [11008]          f32
  out:     [4, 2048, 11008] f32

Sharding: column-parallel (tensor-parallel over out_features) across 8
NeuronCores.  OUT is padded 11008 -> 11264 = 8 * 1408 so every core gets
11 full 128-row tiles.  x is replicated to every core.

All activation/weight preprocessing happens on the host: x is cast to
bf16 and laid out as K-major lhsT tiles xt[m, p, u, t] = x[128m+t, 128u+p];
the weight shard is dequantized to bf16 and transposed to
wt[p, u, o] = w[o, 128u+p].  The device kernel is then a pure streaming
GEMM: resident weights + bias, stream x tiles in, 32 accumulating bf16
matmuls per (token-tile, out-block) into PSUM f32 (k-chunk-outer order so
the PE only ever waits for the earliest unarrived weight chunk), DVE adds
bias on PSUM->SBUF evict, DMA the f32 result out.
"""

import numpy as np

import concourse.bass as bass
import concourse.mybir as mybir
import concourse.tile as tile
from concourse import bacc
from concourse.bass_utils import run_bass_kernel_spmd

P = 128
B, S, IN, OUT, G = 4, 2048, 4096, 11008, 256
NCORES = 8
T = B * S                      # 8192 tokens
OUT_PAD = ((OUT + NCORES * P - 1) // (NCORES * P)) * (NCORES * P)  # 11264
OUT_SH = OUT_PAD // NCORES     # 1408 out features per core
NG = IN // G                   # 16 quant groups per row
KT = IN // P                   # 32 k-tiles
MT = T // P                    # 64 token tiles
UCH = 4                        # k-tiles per resident weight chunk
XSPLIT = 1                     # sub-DMAs per x tile
XU = KT // XSPLIT              # k-tiles per x sub-tile
F32 = mybir.dt.float32
BF16 = mybir.dt.bfloat16

NBLK = [(0, 512), (512, 512), (1024, OUT_SH - 1024)]


def emit_kernel(tc, nc, xt_d, wt_d, bb_d, y_d):
    """Per-core kernel IR.

    xt_d: [MT, P, KT, P] bf16  (host-transposed lhsT tiles, replicated)
    wt_d: [P, KT, OUT_SH] bf16 (host-dequantized transposed weight shard)
    bb_d: [P, OUT_SH]     f32  (bias shard broadcast along partitions)
    y_d:  [T, OUT_SH]     f32  (output shard)
    """
    from contextlib import ExitStack
    ctx = ExitStack()
    const = ctx.enter_context(tc.tile_pool(name="const", bufs=1))
    wtp = ctx.enter_context(tc.tile_pool(name="wt", bufs=1))
    xp = ctx.enter_context(tc.tile_pool(name="x", bufs=4))
    outp = ctx.enter_context(tc.tile_pool(name="out", bufs=3))
    psp = ctx.enter_context(tc.tile_pool(name="psum", bufs=2, space="PSUM"))

    DEPTH = 2

    def produce(m):
        subs = []
        for i in range(XSPLIT):
            xt = xp.tile([P, XU, P], BF16, name=f"xt{i}")
            nc.sync.dma_start(xt[:], xt_d[m, :, i * XU:(i + 1) * XU, :])
            subs.append(xt)
        return subs

    # First x tiles ahead of the weight stream on their own queue.
    xts = {m: produce(m) for m in range(min(DEPTH, MT))}

    biasb = const.tile([P, OUT_SH], F32)
    nc.gpsimd.dma_start(biasb[:], bb_d[:, :])
    # Resident weight chunks; separate tiles so early matmuls only wait on
    # the first chunk.  Spread over three DMA queues.
    # g-order alternation over two queues so chunks arrive in consumption
    # order; sync stays clear for x tiles.
    wts = []
    wq = [nc.scalar, nc.gpsimd]
    for g in range(KT // UCH):
        wtt = wtp.tile([P, UCH, OUT_SH], BF16, name=f"wt{g}")
        wq[g % 2].dma_start(wtt[:], wt_d[:, g * UCH:(g + 1) * UCH, :])
        wts.append(wtt)

    sq = [nc.gpsimd, nc.scalar, nc.sync]

    def evict(m, nb, n0, sz, ps, qi=None):
        t0 = m * P
        ot = outp.tile([P, 512], F32, name=f"ot{nb}")
        nc.vector.tensor_tensor(
            ot[:, :sz], ps, biasb[:, n0:n0 + sz], mybir.AluOpType.add
        )
        sq[(m if qi is None else qi) % 3].dma_start(
            y_d[t0:t0 + P, n0:n0 + sz], ot[:, :sz])

    def alloc_ps():
        return [psp.tile([P, 512], F32, name=f"ps{nb}")[:, :sz]
                for nb, (n0, sz) in enumerate(NBLK)]

    def mm(pss, xtf, u, nb, n0, sz):
        nc.tensor.matmul(
            pss[nb],
            xtf[u // XU][:, u % XU, :],
            wts[u // UCH][:, u % UCH, n0:n0 + sz],
            start=(u == 0),
            stop=(u == KT - 1),
        )

    # Startup: interleave the first two token tiles chunk-by-chunk (6 PSUM
    # banks) so the PE's weight-consumption rate stays below the DMA
    # delivery rate while the resident weight stream lands.
    NSTART = min(2, MT)
    for m in range(NSTART, min(NSTART + DEPTH, MT)):
        xts[m] = produce(m)
    spss = [alloc_ps() for _ in range(NSTART)]
    for u in range(KT):
        for i in range(NSTART):
            for nb, (n0, sz) in enumerate(NBLK):
                mm(spss[i], xts[i], u, nb, n0, sz)
    for i in range(NSTART):
        xts.pop(i)
        for nb, (n0, sz) in enumerate(NBLK):
            evict(i, nb, n0, sz, spss[i][nb])

    for m in range(NSTART, MT):
        if m + DEPTH < MT:
            xts[m + DEPTH] = produce(m + DEPTH)
        xtf = xts.pop(m)
        pss = alloc_ps()
        if m == MT - 1:
            # Last tile: block-sequential so each block's evict+store
            # overlaps the remaining blocks' matmuls, and stores rotated
            # across queues so the final drain flushes in parallel.
            for nb, (n0, sz) in enumerate(NBLK):
                for u in range(KT):
                    mm(pss, xtf, u, nb, n0, sz)
                evict(m, nb, n0, sz, pss[nb], qi=m + nb)
        else:
            # k-chunk-outer so weight chunks are consumed in arrival order.
            for u in range(KT):
                for nb, (n0, sz) in enumerate(NBLK):
                    mm(pss, xtf, u, nb, n0, sz)
            for nb, (n0, sz) in enumerate(NBLK):
                evict(m, nb, n0, sz, pss[nb])

    ctx.close()


def build_nc(debug=False):
    nc = bacc.Bacc(
        "TRN2",
        target_bir_lowering=False,
        debug=debug,
        num_devices=NCORES,
        enable_asserts=debug,
    )
    xt_d = nc.dram_tensor("xt", [MT, P, KT, P], BF16, kind="ExternalInput").ap()
    wt_d = nc.dram_tensor("wt", [P, KT, OUT_SH], BF16, kind="ExternalInput").ap()
    bb_d = nc.dram_tensor("biasb", [P, OUT_SH], F32, kind="ExternalInput").ap()
    y_d = nc.dram_tensor("y", [T, OUT_SH], F32, kind="ExternalOutput").ap()
    with tile.TileContext(nc) as tc:
        emit_kernel(tc, nc, xt_d, wt_d, bb_d, y_d)
    nc.compile()
    return nc


_NC_CACHE = {}


def _get_nc():
    if "nc" not in _NC_CACHE:
        _NC_CACHE["nc"] = build_nc()
    return _NC_CACHE["nc"]


def prep_inputs(x, qweight, scale, bias):
    """Host-side prep. Returns in_maps for run_bass_kernel_spmd."""
    import ml_dtypes
    x = np.asarray(x)
    qw = np.asarray(qweight)
    sc = np.asarray(scale, dtype=np.float32)
    b = np.asarray(bias, dtype=np.float32)

    # xt[m, p, u, t] = x[128m + t, 128u + p], bf16
    x2 = x.reshape(T, IN).astype(ml_dtypes.bfloat16)
    xt = np.ascontiguousarray(
        x2.reshape(MT, P, KT, P).transpose(0, 3, 2, 1))

    # Dequantize exactly as the reference does (q / scale, f32), then bf16.
    qw2 = qw.reshape(OUT, NG, G)
    w = (qw2.astype(np.float32) / sc.reshape(OUT, NG, 1)).reshape(OUT, IN)
    w_p = np.zeros((OUT_PAD, IN), dtype=ml_dtypes.bfloat16)
    w_p[:OUT] = w.astype(ml_dtypes.bfloat16)
    b_p = np.zeros(OUT_PAD, dtype=np.float32)
    b_p[:OUT] = b

    in_maps = []
    for c in range(NCORES):
        sl = slice(c * OUT_SH, (c + 1) * OUT_SH)
        # wt[p, u, o] = w[o, 128u + p]
        wt = np.ascontiguousarray(
            w_p[sl].reshape(OUT_SH, KT, P).transpose(2, 1, 0))
        in_maps.append({
            "xt": xt,
            "wt": wt,
            "biasb": np.ascontiguousarray(
                np.broadcast_to(b_p[sl][None, :], (P, OUT_SH))
            ),
        })
    return in_maps


def run(x, qweight, scale, bias, trace=False):
    nc = _get_nc()
    in_maps = prep_inputs(x, qweight, scale, bias)
    res = run_bass_kernel_spmd(nc, in_maps, core_ids=list(range(NCORES)),
                               trace=trace)
    ys = [np.asarray(res.results[c]["y"]) for c in range(NCORES)]
    out = np.concatenate(ys, axis=1)[:, :OUT]
    return out.reshape(B, S, OUT).astype(np.float32, copy=False), res


def kernel(x, qweight, scale, bias):
    out, _ = run(x, qweight, scale, bias, trace=False)
    return out


# revision 12
# speedup vs baseline: 1.0320x; 1.0291x over previous
"""Trainium2 Bass kernel for nn_CLinear (group-quantized linear layer).

Computes out = x @ dequant(qweight).T + bias where
  x:       [4, 2048, 4096] f32
  qweight: [11008, 16, 256] int8 (group-quantized, G=256)
  scale:   [11008, 16, 1]   f32  (w = qweight / scale)
  bias:    [11008]          f32
  out:     [4, 2048, 11008] f32

Sharding: column-parallel (tensor-parallel over out_features) across 8
NeuronCores.  OUT is padded 11008 -> 11264 = 8 * 1408 so every core gets
11 full 128-row tiles.  x is replicated to every core.

All activation/weight preprocessing happens on the host: x is cast to
bf16 and laid out as K-major lhsT tiles xt[m, p, u, t] = x[128m+t, 128u+p];
the weight shard is dequantized to bf16 and transposed to
wt[p, u, o] = w[o, 128u+p].  The device kernel is then a pure streaming
GEMM: resident weights + bias, stream x tiles in, 32 accumulating bf16
matmuls per (token-tile, out-block) into PSUM f32 (k-chunk-outer order so
the PE only ever waits for the earliest unarrived weight chunk), DVE adds
bias on PSUM->SBUF evict, DMA the f32 result out.
"""

import numpy as np

import concourse.bass as bass
import concourse.mybir as mybir
import concourse.tile as tile
from concourse import bacc
from concourse.bass_utils import run_bass_kernel_spmd

P = 128
B, S, IN, OUT, G = 4, 2048, 4096, 11008, 256
NCORES = 8
T = B * S                      # 8192 tokens
OUT_PAD = OUT                  # 11008 = 8 * 1376, no padding needed
OUT_SH = OUT_PAD // NCORES     # 1376 out features per core (512+512+352)
NG = IN // G                   # 16 quant groups per row
KT = IN // P                   # 32 k-tiles
MT = T // P                    # 64 token tiles
UCH = 4                        # k-tiles per resident weight chunk
XSPLIT = 1                     # sub-DMAs per x tile
XU = KT // XSPLIT              # k-tiles per x sub-tile
F32 = mybir.dt.float32
BF16 = mybir.dt.bfloat16

NBLK = [(0, 512), (512, 512), (1024, OUT_SH - 1024)]


def emit_kernel(tc, nc, xt_d, wt_d, bb_d, y_d):
    """Per-core kernel IR.

    xt_d: [MT, P, KT, P] bf16  (host-transposed lhsT tiles, replicated)
    wt_d: [P, KT, OUT_SH] bf16 (host-dequantized transposed weight shard)
    bb_d: [P, OUT_SH]     f32  (bias shard broadcast along partitions)
    y_d:  [T, OUT_SH]     f32  (output shard)
    """
    from contextlib import ExitStack
    ctx = ExitStack()
    const = ctx.enter_context(tc.tile_pool(name="const", bufs=1))
    wtp = ctx.enter_context(tc.tile_pool(name="wt", bufs=1))
    xp = ctx.enter_context(tc.tile_pool(name="x", bufs=4))
    outp = ctx.enter_context(tc.tile_pool(name="out", bufs=3))
    psp = ctx.enter_context(tc.tile_pool(name="psum", bufs=2, space="PSUM"))

    DEPTH = 2

    def produce(m):
        subs = []
        for i in range(XSPLIT):
            xt = xp.tile([P, XU, P], BF16, name=f"xt{i}")
            nc.sync.dma_start(xt[:], xt_d[m, :, i * XU:(i + 1) * XU, :])
            subs.append(xt)
        return subs

    # First x tiles ahead of the weight stream on their own queue.
    xts = {m: produce(m) for m in range(min(DEPTH, MT))}

    biasb = const.tile([P, OUT_SH], F32)
    nc.gpsimd.dma_start(biasb[:], bb_d[:, :])
    # Resident weight chunks; separate tiles so early matmuls only wait on
    # the first chunk.  Spread over three DMA queues.
    # g-order alternation over two queues so chunks arrive in consumption
    # order; sync stays clear for x tiles.
    wts = []
    wq = [nc.scalar, nc.gpsimd]
    for g in range(KT // UCH):
        wtt = wtp.tile([P, UCH, OUT_SH], BF16, name=f"wt{g}")
        wq[g % 2].dma_start(wtt[:], wt_d[:, g * UCH:(g + 1) * UCH, :])
        wts.append(wtt)

    sq = [nc.gpsimd, nc.scalar, nc.sync]

    def evict(m, nb, n0, sz, ps, qi=None):
        t0 = m * P
        ot = outp.tile([P, 512], F32, name=f"ot{nb}")
        nc.vector.tensor_tensor(
            ot[:, :sz], ps, biasb[:, n0:n0 + sz], mybir.AluOpType.add
        )
        sq[(m if qi is None else qi) % 3].dma_start(
            y_d[t0:t0 + P, n0:n0 + sz], ot[:, :sz])

    def alloc_ps():
        return [psp.tile([P, 512], F32, name=f"ps{nb}")[:, :sz]
                for nb, (n0, sz) in enumerate(NBLK)]

    def mm(pss, xtf, u, nb, n0, sz):
        nc.tensor.matmul(
            pss[nb],
            xtf[u // XU][:, u % XU, :],
            wts[u // UCH][:, u % UCH, n0:n0 + sz],
            start=(u == 0),
            stop=(u == KT - 1),
        )

    # Startup: interleave the first two token tiles chunk-by-chunk (6 PSUM
    # banks) so the PE's weight-consumption rate stays below the DMA
    # delivery rate while the resident weight stream lands.
    NSTART = min(2, MT)
    for m in range(NSTART, min(NSTART + DEPTH, MT)):
        xts[m] = produce(m)
    spss = [alloc_ps() for _ in range(NSTART)]
    for u in range(KT):
        for i in range(NSTART):
            for nb, (n0, sz) in enumerate(NBLK):
                mm(spss[i], xts[i], u, nb, n0, sz)
    for i in range(NSTART):
        xts.pop(i)
        for nb, (n0, sz) in enumerate(NBLK):
            evict(i, nb, n0, sz, spss[i][nb])

    for m in range(NSTART, MT):
        if m + DEPTH < MT:
            xts[m + DEPTH] = produce(m + DEPTH)
        xtf = xts.pop(m)
        pss = alloc_ps()
        if m == MT - 1:
            # Last tile: block-sequential so each block's evict+store
            # overlaps the remaining blocks' matmuls, and stores rotated
            # across queues so the final drain flushes in parallel.
            for nb, (n0, sz) in enumerate(NBLK):
                for u in range(KT):
                    mm(pss, xtf, u, nb, n0, sz)
                evict(m, nb, n0, sz, pss[nb], qi=m + nb)
        else:
            # k-chunk-outer so weight chunks are consumed in arrival order.
            for u in range(KT):
                for nb, (n0, sz) in enumerate(NBLK):
                    mm(pss, xtf, u, nb, n0, sz)
            for nb, (n0, sz) in enumerate(NBLK):
                evict(m, nb, n0, sz, pss[nb])

    ctx.close()


def build_nc(debug=False):
    nc = bacc.Bacc(
        "TRN2",
        target_bir_lowering=False,
        debug=debug,
        num_devices=NCORES,
        enable_asserts=debug,
    )
    xt_d = nc.dram_tensor("xt", [MT, P, KT, P], BF16, kind="ExternalInput").ap()
    wt_d = nc.dram_tensor("wt", [P, KT, OUT_SH], BF16, kind="ExternalInput").ap()
    bb_d = nc.dram_tensor("biasb", [P, OUT_SH], F32, kind="ExternalInput").ap()
    y_d = nc.dram_tensor("y", [T, OUT_SH], F32, kind="ExternalOutput").ap()
    with tile.TileContext(nc) as tc:
        emit_kernel(tc, nc, xt_d, wt_d, bb_d, y_d)
    nc.compile()
    return nc


_NC_CACHE = {}


def _get_nc():
    if "nc" not in _NC_CACHE:
        _NC_CACHE["nc"] = build_nc()
    return _NC_CACHE["nc"]


def prep_inputs(x, qweight, scale, bias):
    """Host-side prep. Returns in_maps for run_bass_kernel_spmd."""
    import ml_dtypes
    x = np.asarray(x)
    qw = np.asarray(qweight)
    sc = np.asarray(scale, dtype=np.float32)
    b = np.asarray(bias, dtype=np.float32)

    # xt[m, p, u, t] = x[128m + t, 128u + p], bf16
    x2 = x.reshape(T, IN).astype(ml_dtypes.bfloat16)
    xt = np.ascontiguousarray(
        x2.reshape(MT, P, KT, P).transpose(0, 3, 2, 1))

    # Dequantize exactly as the reference does (q / scale, f32), then bf16.
    qw2 = qw.reshape(OUT, NG, G)
    w = (qw2.astype(np.float32) / sc.reshape(OUT, NG, 1)).reshape(OUT, IN)
    w_p = np.zeros((OUT_PAD, IN), dtype=ml_dtypes.bfloat16)
    w_p[:OUT] = w.astype(ml_dtypes.bfloat16)
    b_p = np.zeros(OUT_PAD, dtype=np.float32)
    b_p[:OUT] = b

    in_maps = []
    for c in range(NCORES):
        sl = slice(c * OUT_SH, (c + 1) * OUT_SH)
        # wt[p, u, o] = w[o, 128u + p]
        wt = np.ascontiguousarray(
            w_p[sl].reshape(OUT_SH, KT, P).transpose(2, 1, 0))
        in_maps.append({
            "xt": xt,
            "wt": wt,
            "biasb": np.ascontiguousarray(
                np.broadcast_to(b_p[sl][None, :], (P, OUT_SH))
            ),
        })
    return in_maps


def run(x, qweight, scale, bias, trace=False):
    nc = _get_nc()
    in_maps = prep_inputs(x, qweight, scale, bias)
    res = run_bass_kernel_spmd(nc, in_maps, core_ids=list(range(NCORES)),
                               trace=trace)
    ys = [np.asarray(res.results[c]["y"]) for c in range(NCORES)]
    out = np.concatenate(ys, axis=1)[:, :OUT]
    return out.reshape(B, S, OUT).astype(np.float32, copy=False), res


def kernel(x, qweight, scale, bias):
    out, _ = run(x, qweight, scale, bias, trace=False)
    return out


# revision 17
# speedup vs baseline: 1.0333x; 1.0013x over previous
"""Trainium2 Bass kernel for nn_CLinear (group-quantized linear layer).

Computes out = x @ dequant(qweight).T + bias where
  x:       [4, 2048, 4096] f32
  qweight: [11008, 16, 256] int8 (group-quantized, G=256)
  scale:   [11008, 16, 1]   f32  (w = qweight / scale)
  bias:    [11008]          f32
  out:     [4, 2048, 11008] f32

Sharding: column-parallel (tensor-parallel over out_features) across 8
NeuronCores.  OUT is padded 11008 -> 11264 = 8 * 1408 so every core gets
11 full 128-row tiles.  x is replicated to every core.

All activation/weight preprocessing happens on the host: x is cast to
bf16 and laid out as K-major lhsT tiles xt[m, p, u, t] = x[128m+t, 128u+p];
the weight shard is dequantized to bf16 and transposed to
wt[p, u, o] = w[o, 128u+p].  The device kernel is then a pure streaming
GEMM: resident weights + bias, stream x tiles in, 32 accumulating bf16
matmuls per (token-tile, out-block) into PSUM f32 (k-chunk-outer order so
the PE only ever waits for the earliest unarrived weight chunk), DVE adds
bias on PSUM->SBUF evict, DMA the f32 result out.
"""

import numpy as np

import concourse.bass as bass
import concourse.mybir as mybir
import concourse.tile as tile
from concourse import bacc
from concourse.bass_utils import run_bass_kernel_spmd

P = 128
B, S, IN, OUT, G = 4, 2048, 4096, 11008, 256
NCORES = 8
T = B * S                      # 8192 tokens
OUT_PAD = OUT                  # 11008 = 8 * 1376, no padding needed
OUT_SH = OUT_PAD // NCORES     # 1376 out features per core (512+512+352)
NG = IN // G                   # 16 quant groups per row
KT = IN // P                   # 32 k-tiles
MT = T // P                    # 64 token tiles
UCH = 4                        # k-tiles per resident weight chunk
XSPLIT = 1                     # sub-DMAs per x tile
XU = KT // XSPLIT              # k-tiles per x sub-tile
F32 = mybir.dt.float32
BF16 = mybir.dt.bfloat16

NBLK = [(0, 512), (512, 512), (1024, OUT_SH - 1024)]


def emit_kernel(tc, nc, xt_d, wt_d, bb_d, y_d):
    """Per-core kernel IR.

    xt_d: [MT, P, KT, P] bf16  (host-transposed lhsT tiles, replicated)
    wt_d: [P, KT, OUT_SH] bf16 (host-dequantized transposed weight shard)
    bb_d: [P, OUT_SH]     f32  (bias shard broadcast along partitions)
    y_d:  [T, OUT_SH]     f32  (output shard)
    """
    from contextlib import ExitStack
    ctx = ExitStack()
    const = ctx.enter_context(tc.tile_pool(name="const", bufs=1))
    wtp = ctx.enter_context(tc.tile_pool(name="wt", bufs=1))
    xp = ctx.enter_context(tc.tile_pool(name="x", bufs=5))
    outp = ctx.enter_context(tc.tile_pool(name="out", bufs=3))
    psp = ctx.enter_context(tc.tile_pool(name="psum", bufs=2, space="PSUM"))
    psx = ctx.enter_context(tc.tile_pool(name="psumx", bufs=1, space="PSUM"))

    def produce(m):
        subs = []
        for i in range(XSPLIT):
            xt = xp.tile([P, XU, P], BF16, name=f"xt{i}")
            nc.sync.dma_start(xt[:], xt_d[m, :, i * XU:(i + 1) * XU, :])
            subs.append(xt)
        return subs

    # First x tiles ahead of the weight stream on their own queue.
    xts = {m: produce(m) for m in range(min(3, MT))}

    # Resident weight chunks; separate tiles so early matmuls only wait on
    # the first chunk.  Balanced over the three HWDGE queues roughly in
    # consumption order: scalar g0,g2,g4,g6; gpsimd g1,g3,g5; sync g7
    # (behind the first x tiles).
    wts = []
    wq = {0: nc.scalar, 2: nc.scalar, 4: nc.scalar, 6: nc.scalar,
          1: nc.gpsimd, 3: nc.gpsimd, 5: nc.gpsimd, 7: nc.sync}
    for g in range(KT // UCH):
        wtt = wtp.tile([P, UCH, OUT_SH], BF16, name=f"wt{g}")
        wq[g].dma_start(wtt[:], wt_d[:, g * UCH:(g + 1) * UCH, :])
        wts.append(wtt)
    # Bias last on gpsimd: first needed at the first evict (~25us in).
    biasb = const.tile([P, OUT_SH], F32)
    nc.gpsimd.dma_start(biasb[:], bb_d[:, :])

    sq = [nc.gpsimd, nc.scalar, nc.sync]

    def evict(m, nb, n0, sz, ps, qi=None):
        t0 = m * P
        ot = outp.tile([P, 512], F32, name=f"ot{nb}")
        nc.vector.tensor_tensor(
            ot[:, :sz], ps, biasb[:, n0:n0 + sz], mybir.AluOpType.add
        )
        sq[(m if qi is None else qi) % 3].dma_start(
            y_d[t0:t0 + P, n0:n0 + sz], ot[:, :sz])

    def alloc_ps():
        return [psp.tile([P, 512], F32, name=f"ps{nb}")[:, :sz]
                for nb, (n0, sz) in enumerate(NBLK)]

    def mm(pss, xtf, u, nb, n0, sz):
        nc.tensor.matmul(
            pss[nb],
            xtf[u // XU][:, u % XU, :],
            wts[u // UCH][:, u % UCH, n0:n0 + sz],
            start=(u == 0),
            stop=(u == KT - 1),
        )

    # Startup: interleave the first 2 2/3 token tiles chunk-by-chunk across
    # all 8 PSUM banks so the PE's weight-consumption rate stays below the
    # DMA delivery rate while the resident weight stream lands.  Tile 2's
    # last block is deferred to the main loop (bank budget).
    for m in range(3, min(5, MT)):
        xts[m] = produce(m)
    nextp = min(5, MT)
    spss = [alloc_ps() for _ in range(2)]
    sps2 = [psx.tile([P, 512], F32, name=f"xps{nb}")[:, :sz]
            for nb, (n0, sz) in enumerate(NBLK[:2])]
    for u in range(KT):
        for i in range(2):
            for nb, (n0, sz) in enumerate(NBLK):
                mm(spss[i], xts[i], u, nb, n0, sz)
        for nb, (n0, sz) in enumerate(NBLK[:2]):
            mm(sps2, xts[2], u, nb, n0, sz)
    for i in range(2):
        xts.pop(i)
        for nb, (n0, sz) in enumerate(NBLK):
            evict(i, nb, n0, sz, spss[i][nb])

    for m in range(2, MT):
        if nextp < MT:
            xts[nextp] = produce(nextp)
            nextp += 1
        xtf = xts.pop(m)
        if m == 2:
            # Finish tile 2: only its last block remains, then evict all.
            nb2, (n20, s2z) = 2, NBLK[2]
            ps2 = psp.tile([P, 512], F32, name="ps2")[:, :s2z]
            for u in range(KT):
                mm([None, None, ps2], xtf, u, nb2, n20, s2z)
            for nb, (n0, sz) in enumerate(NBLK[:2]):
                evict(m, nb, n0, sz, sps2[nb])
            evict(m, nb2, n20, s2z, ps2)
            continue
        pss = alloc_ps()
        if m == MT - 1:
            # Last tile: block-sequential so each block's evict+store
            # overlaps the remaining blocks' matmuls, and stores rotated
            # across queues so the final drain flushes in parallel.
            for nb, (n0, sz) in enumerate(NBLK):
                for u in range(KT):
                    mm(pss, xtf, u, nb, n0, sz)
                evict(m, nb, n0, sz, pss[nb], qi=m + nb)
        else:
            # k-chunk-outer so weight chunks are consumed in arrival order.
            for u in range(KT):
                for nb, (n0, sz) in enumerate(NBLK):
                    mm(pss, xtf, u, nb, n0, sz)
            for nb, (n0, sz) in enumerate(NBLK):
                evict(m, nb, n0, sz, pss[nb])

    ctx.close()


def build_nc(debug=False):
    nc = bacc.Bacc(
        "TRN2",
        target_bir_lowering=False,
        debug=debug,
        num_devices=NCORES,
        enable_asserts=debug,
    )
    xt_d = nc.dram_tensor("xt", [MT, P, KT, P], BF16, kind="ExternalInput").ap()
    wt_d = nc.dram_tensor("wt", [P, KT, OUT_SH], BF16, kind="ExternalInput").ap()
    bb_d = nc.dram_tensor("biasb", [P, OUT_SH], F32, kind="ExternalInput").ap()
    y_d = nc.dram_tensor("y", [T, OUT_SH], F32, kind="ExternalOutput").ap()
    with tile.TileContext(nc) as tc:
        emit_kernel(tc, nc, xt_d, wt_d, bb_d, y_d)
    nc.compile()
    return nc


_NC_CACHE = {}


def _get_nc():
    if "nc" not in _NC_CACHE:
        _NC_CACHE["nc"] = build_nc()
    return _NC_CACHE["nc"]


def prep_inputs(x, qweight, scale, bias):
    """Host-side prep. Returns in_maps for run_bass_kernel_spmd."""
    import ml_dtypes
    x = np.asarray(x)
    qw = np.asarray(qweight)
    sc = np.asarray(scale, dtype=np.float32)
    b = np.asarray(bias, dtype=np.float32)

    # xt[m, p, u, t] = x[128m + t, 128u + p], bf16
    x2 = x.reshape(T, IN).astype(ml_dtypes.bfloat16)
    xt = np.ascontiguousarray(
        x2.reshape(MT, P, KT, P).transpose(0, 3, 2, 1))

    # Dequantize exactly as the reference does (q / scale, f32), then bf16.
    qw2 = qw.reshape(OUT, NG, G)
    w = (qw2.astype(np.float32) / sc.reshape(OUT, NG, 1)).reshape(OUT, IN)
    w_p = np.zeros((OUT_PAD, IN), dtype=ml_dtypes.bfloat16)
    w_p[:OUT] = w.astype(ml_dtypes.bfloat16)
    b_p = np.zeros(OUT_PAD, dtype=np.float32)
    b_p[:OUT] = b

    in_maps = []
    for c in range(NCORES):
        sl = slice(c * OUT_SH, (c + 1) * OUT_SH)
        # wt[p, u, o] = w[o, 128u + p]
        wt = np.ascontiguousarray(
            w_p[sl].reshape(OUT_SH, KT, P).transpose(2, 1, 0))
        in_maps.append({
            "xt": xt,
            "wt": wt,
            "biasb": np.ascontiguousarray(
                np.broadcast_to(b_p[sl][None, :], (P, OUT_SH))
            ),
        })
    return in_maps


def run(x, qweight, scale, bias, trace=False):
    nc = _get_nc()
    in_maps = prep_inputs(x, qweight, scale, bias)
    res = run_bass_kernel_spmd(nc, in_maps, core_ids=list(range(NCORES)),
                               trace=trace)
    ys = [np.asarray(res.results[c]["y"]) for c in range(NCORES)]
    out = np.concatenate(ys, axis=1)[:, :OUT]
    return out.reshape(B, S, OUT).astype(np.float32, copy=False), res


def kernel(x, qweight, scale, bias):
    out, _ = run(x, qweight, scale, bias, trace=False)
    return out
